# revision 1
# baseline (speedup 1.0000x reference)
"""Trainium2 Bass kernel for nn_Linear_67070209294813 (moe_routing).

Computes, for x:[B,S,Din] f32:
    base = x @ w_base.T + b_base
    gate = softmax(blend(x @ w_router_{img,text}.T + b_router), axis=E)
    h    = einsum("td,erd->ter", x, lora_A) * gate
    out  = base + einsum("ter,eor->to", h, lora_B) * SCALING

Strategy: data-parallel over the 8192 tokens across 8 NeuronCores (1024
tokens/core).  Per core one fp32r (full-rate fp32) GEMM out^T[dout, tok] =
sum_k wT[k,dout-tile].T @ xT[k, tok] with the LoRA rank-65 (64 rank dims +
1 bias row) matmul accumulated into the same PSUM banks, so bias add and
the base+lora sum cost nothing.  Routers/LoRA-A run as one small
[din,72]-wide matmul; softmax runs in token-partition layout via two tiny
PE transposes; the gate is expanded over the 16 ranks of each expert with
a 0/1 replication matmul.

All operands are pre-transposed / blocked on the host so every DMA is
contiguous and the contraction dim lands on SBUF partitions.
"""

import sys

sys.path.insert(0, "/opt/trn_rl_repo")

import numpy as np

import concourse.bass as bass  # noqa: F401  (bass must import before tile)
import concourse.mybir as mybir
import concourse.tile as tile
from concourse import bacc
from concourse.bass_utils import run_bass_kernel_spmd

B, S, D_IN, D_OUT = 4, 2048, 4096, 4096
R, E, SPLIT = 16, 4, 32
SCALING = 32.0 / 16.0
N_CORES = 8
TOK = B * S
TPC = TOK // N_CORES  # tokens per core
ER = E * R  # 64 rank dims across experts

F32 = mybir.dt.float32
F32R = mybir.dt.float32r
AF = mybir.ActivationFunctionType


def build_program(din, dout, tpc):
    """Emit + compile the per-core Tile program. Returns the Bacc object."""
    nk = din // 128  # k tiles (contraction)
    nm = dout // 128  # output-row tiles
    nt = tpc // 128  # token chunks (for the tiny softmax transposes)
    # moving-dim slices of the token axis (<=512 fp32 per matmul)
    n_sl = [(i, min(512, tpc - i)) for i in range(0, tpc, 512)]
    wr = 2 * E  # router logit columns (img then text)
    hcols = ER + wr  # 72: lora-A ranks + both routers

    nc = bacc.Bacc("TRN2", target_bir_lowering=False, debug=False)

    xT = nc.dram_tensor("xT", [din, tpc], F32R, kind="ExternalInput").ap()
    wblk = nc.dram_tensor("wblk", [nm * nk, 128, 128], F32R, kind="ExternalInput").ap()
    ar = nc.dram_tensor("ar", [din, hcols], F32R, kind="ExternalInput").ap()
    bf = nc.dram_tensor("bf", [ER + 1, dout], F32R, kind="ExternalInput").ap()
    r4 = nc.dram_tensor("r4", [E, ER], F32R, kind="ExternalInput").ap()
    ones = nc.dram_tensor("ones", [1, tpc], F32R, kind="ExternalInput").ap()
    ident = nc.dram_tensor("ident", [128, 128], F32, kind="ExternalInput").ap()
    mask = nc.dram_tensor("mask", [128, nt], F32, kind="ExternalInput").ap()
    bbl = nc.dram_tensor("bbl", [128, nt * E], F32, kind="ExternalInput").ap()
    outT = nc.dram_tensor("outT", [dout, tpc], F32, kind="ExternalOutput").ap()

    with tile.TileContext(nc) as tc:
        with (
            tc.tile_pool(name="big", bufs=1) as big,
            tc.tile_pool(name="const", bufs=1) as const,
            tc.tile_pool(name="wp", bufs=4) as wp,
            tc.tile_pool(name="outp", bufs=2) as outp,
            tc.tile_pool(name="small", bufs=1) as small,
            tc.tile_pool(name="ps_main", bufs=2, space="PSUM") as ps_main,
            tc.tile_pool(name="ps_h", bufs=1, space="PSUM") as ps_h,
            tc.tile_pool(name="ps_t", bufs=2, space="PSUM") as ps_t,
        ):
            # ---- constants + x load -------------------------------------
            xt = big.tile([128, nk * tpc], F32R)
            for k in range(nk):
                nc.sync.dma_start(
                    xt[:, k * tpc : (k + 1) * tpc], xT[k * 128 : (k + 1) * 128, :]
                )
            ar_sb = const.tile([128, nk * hcols], F32R)
            nc.sync.dma_start(
                ar_sb[:].rearrange("p (k j) -> p k j", j=hcols),
                ar.rearrange("(k p) j -> p k j", p=128),
            )
            bf_sb = const.tile([ER + 1, dout], F32R)
            nc.sync.dma_start(bf_sb[:], bf[:, :])
            r4_sb = const.tile([E, ER], F32R)
            nc.sync.dma_start(r4_sb[:], r4[:, :])
            id_sb = const.tile([128, 128], F32)
            nc.sync.dma_start(id_sb[:], ident[:, :])
            mask_sb = const.tile([128, nt], F32)
            nc.sync.dma_start(mask_sb[:], mask[:, :])
            bbl_sb = const.tile([128, nt * E], F32)
            nc.sync.dma_start(bbl_sb[:], bbl[:, :])

            # ---- phase B: hT[er,tok] + router logits, one wide matmul ---
            ph = ps_h.tile([hcols, tpc], F32, tag="h")
            for k in range(nk):
                lhs = ar_sb[:, k * hcols : (k + 1) * hcols]
                for o, w_ in n_sl:
                    nc.tensor.matmul(
                        ph[:, o : o + w_],
                        lhs,
                        xt[:, k * tpc + o : k * tpc + o + w_],
                        start=(k == 0),
                        stop=(k == nk - 1),
                    )
            hT = small.tile([hcols, tpc], F32)
            nc.vector.tensor_copy(hT[:], ph[:])
            lgT = small.tile([wr, tpc], F32)
            # partition-moving copy (rows ER..ER+wr -> 0..wr) must be a DMA
            nc.sync.dma_start(lgT[:], hT[ER : ER + wr, :])

            def base_kloop(m):
                ps = ps_main.tile([128, tpc], F32, tag="ps")
                for k in range(nk):
                    wt = wp.tile([128, 128], F32R, tag="w")
                    nc.sync.dma_start(wt[:], wblk[m * nk + k, :, :])
                    for o, w_ in n_sl:
                        nc.tensor.matmul(
                            ps[:, o : o + w_],
                            wt[:],
                            xt[:, k * tpc + o : k * tpc + o + w_],
                            start=(k == 0),
                            stop=False,
                        )
                return ps

            def lora_tail(m, ps, hw):
                for o, w_ in n_sl:
                    nc.tensor.matmul(
                        ps[:, o : o + w_],
                        bf_sb[:, m * 128 : (m + 1) * 128],
                        hw[:, o : o + w_],
                        start=False,
                        stop=True,
                    )
                ot = outp.tile([128, tpc], F32, tag="o")
                nc.vector.tensor_copy(ot[:], ps[:])
                nc.sync.dma_start(outT[m * 128 : (m + 1) * 128, :], ot[:])

            # m=0 base matmuls first so the PE stays busy while the (DVE)
            # softmax below runs; its lora tail is emitted after gating.
            ps0 = base_kloop(0)

            # ---- gating: softmax over E in token-partition layout -------
            lg = small.tile([128, nt * wr], F32)
            for t in range(nt):
                pt = ps_t.tile([128, wr], F32, tag="tp")
                nc.tensor.transpose(
                    pt[:], lgT[:, t * 128 : (t + 1) * 128], id_sb[0:wr, 0:wr]
                )
                nc.vector.tensor_copy(lg[:, t * wr : (t + 1) * wr], pt[:])
            lg3 = lg[:].rearrange("p (t j) -> p t j", j=wr)
            l_img, l_text = lg3[:, :, 0:E], lg3[:, :, E : 2 * E]
            g = small.tile([128, nt * E], F32)
            g3 = g[:].rearrange("p (t e) -> p t e", e=E)
            mb = mask_sb[:, :, None].broadcast_to([128, nt, E])
            nc.vector.tensor_sub(g3, l_img, l_text)
            nc.vector.tensor_mul(g3, g3, mb)
            nc.vector.tensor_add(g3, g3, l_text)
            nc.vector.tensor_add(g[:], g[:], bbl_sb[:])
            nc.scalar.activation(g[:], g[:], AF.Exp)
            zt = small.tile([128, nt], F32)
            nc.vector.reduce_sum(zt[:], g3, axis=mybir.AxisListType.X)
            nc.vector.reciprocal(zt[:], zt[:])
            nc.vector.tensor_mul(g3, g3, zt[:, :, None].broadcast_to([128, nt, E]))
            # gate back to [E, tok] layout, then expand across the 16 ranks
            g4 = small.tile([E, tpc], F32R)
            for t in range(nt):
                pt = ps_t.tile([E, 128], F32, tag="tp")
                nc.tensor.transpose(pt[:], g[:, t * E : (t + 1) * E], id_sb[:, :])
                nc.vector.tensor_copy(g4[:, t * 128 : (t + 1) * 128], pt[:])
            pgr = ps_h.tile([ER, tpc], F32, tag="h")
            for o, w_ in n_sl:
                nc.tensor.matmul(
                    pgr[:, o : o + w_], r4_sb[:], g4[:, o : o + w_],
                    start=True, stop=True,
                )
            hw = small.tile([ER + 1, tpc], F32R)
            nc.sync.dma_start(hw[ER : ER + 1, :], ones[:, :])
            nc.vector.tensor_mul(hw[0:ER, :], hT[0:ER, :], pgr[:])

            # ---- main GEMM over output-row tiles ------------------------
            lora_tail(0, ps0, hw)
            for m in range(1, nm):
                ps = base_kloop(m)
                lora_tail(m, ps, hw)

    nc.compile()
    return nc


def pack_inputs(
    x_flat, w_base, b_base, w_router_img, b_router_img, w_router_text,
    b_router_text, lora_A, lora_B, n_cores,
):
    """Host-side marshalling into the per-core DRAM layouts."""
    tok, din = x_flat.shape
    dout = w_base.shape[0]
    tpc = tok // n_cores
    nk, nm, nt = din // 128, dout // 128, tpc // 128
    e, r = lora_A.shape[0], lora_A.shape[1]
    er = e * r

    f32 = np.float32
    wT = np.ascontiguousarray(w_base.T)  # [din, dout]
    wblk = np.ascontiguousarray(
        wT.reshape(nk, 128, nm, 128).transpose(2, 0, 1, 3)
    ).reshape(nm * nk, 128, 128)  # tile (m,k) contiguous
    ar = np.concatenate(
        [lora_A.reshape(er, din).T, w_router_img.T, w_router_text.T], axis=1
    ).astype(f32)  # [din, er + 2e]
    bfm = (lora_B.transpose(0, 2, 1).reshape(er, dout) * SCALING).astype(f32)
    bf = np.concatenate([bfm, b_base.reshape(1, dout).astype(f32)], axis=0)
    r4 = np.zeros((e, er), f32)
    for i in range(e):
        r4[i, i * r : (i + 1) * r] = 1.0
    ident = np.eye(128, dtype=f32)

    shared = {
        "wblk": wblk, "ar": ar, "bf": bf, "r4": r4, "ident": ident,
        "ones": np.ones((1, tpc), f32),
    }
    in_maps = []
    for c in range(n_cores):
        sh = x_flat[c * tpc : (c + 1) * tpc]
        xTc = np.ascontiguousarray(sh.T)  # [din, tpc]
        toks = c * tpc + np.arange(tpc)
        m = ((toks % S) < SPLIT).astype(f32)  # image-token mask
        mask_pc = np.ascontiguousarray(m.reshape(nt, 128).T)  # [128, nt]
        bb = (
            m[:, None] * b_router_img[None, :].astype(f32)
            + (1.0 - m[:, None]) * b_router_text[None, :].astype(f32)
        )  # [tpc, e]
        bbl_pc = np.ascontiguousarray(
            bb.reshape(nt, 128, e).transpose(1, 0, 2)
        ).reshape(128, nt * e)
        in_maps.append({"xT": xTc, "mask": mask_pc, "bbl": bbl_pc, **shared})
    return in_maps


_prog_cache = {}


def _get_program():
    key = (D_IN, D_OUT, TPC)
    if key not in _prog_cache:
        _prog_cache[key] = build_program(D_IN, D_OUT, TPC)
    return _prog_cache[key]


def kernel(
    x, w_base, b_base, w_router_img, b_router_img, w_router_text,
    b_router_text, lora_A, lora_B,
):
    x = np.asarray(x, dtype=np.float32)
    x_flat = np.ascontiguousarray(x.reshape(TOK, D_IN))
    in_maps = pack_inputs(
        x_flat, np.asarray(w_base, np.float32), np.asarray(b_base, np.float32),
        np.asarray(w_router_img, np.float32), np.asarray(b_router_img, np.float32),
        np.asarray(w_router_text, np.float32), np.asarray(b_router_text, np.float32),
        np.asarray(lora_A, np.float32), np.asarray(lora_B, np.float32),
        N_CORES,
    )
    nc = _get_program()
    res = run_bass_kernel_spmd(nc, in_maps, core_ids=list(range(N_CORES)))
    out = np.empty((TOK, D_OUT), np.float32)
    for c in range(N_CORES):
        out[c * TPC : (c + 1) * TPC, :] = res.results[c]["outT"].T
    return out.reshape(B, S, D_OUT)



# revision 6
# speedup vs baseline: 1.0061x; 1.0061x over previous
"""Trainium2 Bass kernel for nn_Linear_67070209294813 (moe_routing).

Computes, for x:[B,S,Din] f32:
    base = x @ w_base.T + b_base
    gate = softmax(blend(x @ w_router_{img,text}.T + b_router), axis=E)
    h    = einsum("td,erd->ter", x, lora_A) * gate
    out  = base + einsum("ter,eor->to", h, lora_B) * SCALING

Strategy: data-parallel over the 8192 tokens across 8 NeuronCores (1024
tokens/core).  Per core one GEMM out^T[dout, tok] = sum_k wT-tile.T @
xT[k, tok] with the LoRA rank-65 (64 rank dims + 1 bias row) matmul
accumulated into the same PSUM banks, so bias add and the base+lora sum
cost nothing.  Inputs are cast to bf16 (fp32 PSUM accumulation) which
halves HBM traffic at identical PE throughput; the base weight is
repacked on the host into one contiguous [128, din] block per output
tile so each weight load is a single large DMA.  The router/LoRA-A
matmuls are interleaved into the first base k-loop so the PE paces with
the x-load DMAs; softmax runs in token-partition layout via two tiny PE
transposes; the gate is expanded over the 16 ranks of each expert with
a 0/1 replication matmul.
"""

import sys

sys.path.insert(0, "/opt/trn_rl_repo")

import numpy as np

import concourse.bass as bass  # noqa: F401  (bass must import before tile)
import concourse.mybir as mybir
import concourse.tile as tile
from concourse import bacc
from concourse.bass_utils import run_bass_kernel_spmd

B, S, D_IN, D_OUT = 4, 2048, 4096, 4096
R, E, SPLIT = 16, 4, 32
SCALING = 32.0 / 16.0
N_CORES = 8
TOK = B * S
TPC = TOK // N_CORES  # tokens per core
ER = E * R  # 64 rank dims across experts

USE_BF16 = True

F32 = mybir.dt.float32
MMD = mybir.dt.bfloat16 if USE_BF16 else mybir.dt.float32r
NP_MMD = mybir.dt.np(MMD)
AF = mybir.ActivationFunctionType

# merged f32 const-block column offsets: ident | mask | bbl | r4
_NT = TPC // 128
_CO_ID, _CO_MASK, _CO_BBL, _CO_R4 = 0, 128, 128 + _NT, 128 + _NT + _NT * E
_CW = _CO_R4 + ER


def build_program(din, dout, tpc):
    """Emit + compile the per-core Tile program. Returns the Bacc object."""
    nk = din // 128  # k tiles (contraction)
    nm = dout // 128  # output-row tiles
    nt = tpc // 128  # token chunks (for the tiny softmax transposes)
    # moving-dim slices of the token axis (one PSUM bank = 512 fp32)
    n_sl = [(i, min(512, tpc - i)) for i in range(0, tpc, 512)]
    wr = 2 * E  # router logit columns (img then text)
    hcols = ER + wr  # 72: lora-A ranks + both routers
    xg = 4  # k-tiles per x-load DMA

    nc = bacc.Bacc("TRN2", target_bir_lowering=False, debug=False)

    xT = nc.dram_tensor("xT", [din, tpc], MMD, kind="ExternalInput").ap()
    wlin = nc.dram_tensor("wlin", [nm, 128, nk * 128], MMD, kind="ExternalInput").ap()
    ar = nc.dram_tensor("ar", [din, hcols], MMD, kind="ExternalInput").ap()
    bf = nc.dram_tensor("bf", [ER + 1, dout], MMD, kind="ExternalInput").ap()
    cf = nc.dram_tensor("cf", [128, _CW], F32, kind="ExternalInput").ap()
    outT = nc.dram_tensor("outT", [dout, tpc], F32, kind="ExternalOutput").ap()

    with tile.TileContext(nc) as tc:
        with (
            tc.tile_pool(name="big", bufs=1) as big,
            tc.tile_pool(name="const", bufs=1) as const,
            tc.tile_pool(name="wp", bufs=2) as wp,
            tc.tile_pool(name="outp", bufs=2) as outp,
            tc.tile_pool(name="small", bufs=1) as small,
            tc.tile_pool(name="ps_main", bufs=2, space="PSUM") as ps_main,
            tc.tile_pool(name="ps_h", bufs=1, space="PSUM") as ps_h,
            tc.tile_pool(name="ps_t", bufs=2, space="PSUM") as ps_t,
        ):
            # ---- constants + x load -------------------------------------
            # first x chunk and the m=0 weight block lead the DMA queue so
            # the PE can start as soon as they land.
            xt = big.tile([128, nk * tpc], MMD)
            wt0 = wp.tile([128, nk * 128], MMD, tag="w")
            for g in range(nk // xg):
                nc.sync.dma_start(
                    xt[:, g * xg * tpc : (g + 1) * xg * tpc].rearrange(
                        "p (g t) -> p g t", t=tpc
                    ),
                    xT[g * xg * 128 : (g + 1) * xg * 128, :].rearrange(
                        "(g p) t -> p g t", p=128
                    ),
                )
                if g == 0:
                    nc.sync.dma_start(wt0[:], wlin[0, :, :])
            ar_sb = const.tile([128, nk * hcols], MMD)
            nc.sync.dma_start(
                ar_sb[:].rearrange("p (k j) -> p k j", j=hcols),
                ar.rearrange("(k p) j -> p k j", p=128),
            )
            bf_sb = const.tile([ER + 1, dout], MMD)
            nc.sync.dma_start(bf_sb[:], bf[:, :])
            cf_sb = const.tile([128, _CW], F32)
            nc.sync.dma_start(cf_sb[:], cf[:, :])
            id_sb = cf_sb[:, _CO_ID : _CO_ID + 128]
            mask_sb = cf_sb[:, _CO_MASK : _CO_MASK + nt]
            bbl_sb = cf_sb[:, _CO_BBL : _CO_BBL + nt * E]
            r4_sb = cf_sb[0:E, _CO_R4 : _CO_R4 + ER]

            def base_kloop(m, also_ph=None, wt=None):
                ps = ps_main.tile([128, tpc], F32, tag="ps")
                if wt is None:
                    wt = wp.tile([128, nk * 128], MMD, tag="w")
                    nc.sync.dma_start(wt[:], wlin[m, :, :])
                for k in range(nk):
                    for o, w_ in n_sl:
                        nc.tensor.matmul(
                            ps[:, o : o + w_],
                            wt[:, k * 128 : (k + 1) * 128],
                            xt[:, k * tpc + o : k * tpc + o + w_],
                            start=(k == 0),
                            stop=False,
                        )
                        if also_ph is not None:
                            nc.tensor.matmul(
                                also_ph[:, o : o + w_],
                                ar_sb[:, k * hcols : (k + 1) * hcols],
                                xt[:, k * tpc + o : k * tpc + o + w_],
                                start=(k == 0),
                                stop=(k == nk - 1),
                            )
                return ps

            def lora_tail(m, ps, hw):
                for o, w_ in n_sl:
                    nc.tensor.matmul(
                        ps[:, o : o + w_],
                        bf_sb[:, m * 128 : (m + 1) * 128],
                        hw[:, o : o + w_],
                        start=False,
                        stop=True,
                    )
                ot = outp.tile([128, tpc], F32, tag="o")
                nc.scalar.copy(ot[:], ps[:])
                nc.sync.dma_start(outT[m * 128 : (m + 1) * 128, :], ot[:])

            # m=0 base matmuls with the router/lora-A matmuls interleaved so
            # the PE paces with the x-load DMAs; its lora tail is emitted
            # after gating.
            ph = ps_h.tile([hcols, tpc], F32, tag="h")
            ps0 = base_kloop(0, also_ph=ph, wt=wt0)

            hT = small.tile([hcols, tpc], F32)
            nc.vector.tensor_copy(hT[:], ph[:])
            lgT = small.tile([wr, tpc], F32)
            # partition-moving copy (rows ER..ER+wr -> 0..wr) must be a DMA
            nc.sync.dma_start(lgT[:], hT[ER : ER + wr, :])

            # ---- gating: softmax over E in token-partition layout -------
            lg = small.tile([128, nt * wr], F32)
            for t in range(nt):
                pt = ps_t.tile([128, wr], F32, tag="tp")
                nc.tensor.transpose(
                    pt[:], lgT[:, t * 128 : (t + 1) * 128], id_sb[0:wr, 0:wr]
                )
                nc.vector.tensor_copy(lg[:, t * wr : (t + 1) * wr], pt[:])
            lg3 = lg[:].rearrange("p (t j) -> p t j", j=wr)
            l_img, l_text = lg3[:, :, 0:E], lg3[:, :, E : 2 * E]
            g = small.tile([128, nt * E], F32)
            g3 = g[:].rearrange("p (t e) -> p t e", e=E)
            mb = mask_sb[:, :, None].broadcast_to([128, nt, E])
            nc.vector.tensor_sub(g3, l_img, l_text)
            nc.vector.tensor_mul(g3, g3, mb)
            nc.vector.tensor_add(g3, g3, l_text)
            nc.vector.tensor_add(g[:], g[:], bbl_sb)
            nc.scalar.activation(g[:], g[:], AF.Exp)
            zt = small.tile([128, nt], F32)
            nc.vector.reduce_sum(zt[:], g3, axis=mybir.AxisListType.X)
            nc.vector.reciprocal(zt[:], zt[:])
            nc.vector.tensor_mul(g3, g3, zt[:, :, None].broadcast_to([128, nt, E]))
            # gate back to [E, tok] layout, then expand across the 16 ranks
            g4 = small.tile([E, tpc], F32)
            for t in range(nt):
                pt = ps_t.tile([E, 128], F32, tag="tp")
                nc.tensor.transpose(pt[:], g[:, t * E : (t + 1) * E], id_sb[:, :])
                nc.vector.tensor_copy(g4[:, t * 128 : (t + 1) * 128], pt[:])
            pgr = ps_h.tile([ER, tpc], F32, tag="h")
            for o, w_ in n_sl:
                nc.tensor.matmul(
                    pgr[:, o : o + w_], r4_sb, g4[:, o : o + w_],
                    start=True, stop=True,
                )
            hw = small.tile([ER + 1, tpc], MMD)
            nc.vector.memset(hw[ER : ER + 1, :], 1.0)
            nc.vector.tensor_mul(hw[0:ER, :], hT[0:ER, :], pgr[:])

            # ---- main GEMM over output-row tiles ------------------------
            lora_tail(0, ps0, hw)
            for m in range(1, nm):
                ps = base_kloop(m)
                lora_tail(m, ps, hw)

    nc.compile()
    return nc


def pack_inputs(
    x_flat, w_base, b_base, w_router_img, b_router_img, w_router_text,
    b_router_text, lora_A, lora_B, n_cores,
):
    """Host-side marshalling into the per-core DRAM layouts."""
    tok, din = x_flat.shape
    dout = w_base.shape[0]
    tpc = tok // n_cores
    nk, nm, nt = din // 128, dout // 128, tpc // 128
    e, r = lora_A.shape[0], lora_A.shape[1]
    er = e * r

    f32 = np.float32
    # wlin[m, p, k*128+c] = w_base[m*128+c, k*128+p]
    wlin = np.ascontiguousarray(
        np.asarray(w_base, f32).reshape(nm, 128, nk, 128).transpose(0, 3, 2, 1)
    ).astype(NP_MMD).reshape(nm, 128, nk * 128)
    ar = np.concatenate(
        [lora_A.reshape(er, din).T, w_router_img.T, w_router_text.T], axis=1
    ).astype(NP_MMD)  # [din, er + 2e]
    bfm = (lora_B.transpose(0, 2, 1).reshape(er, dout) * SCALING).astype(f32)
    bf = np.concatenate(
        [bfm, np.asarray(b_base, f32).reshape(1, dout)], axis=0
    ).astype(NP_MMD)

    r4 = np.zeros((e, er), f32)
    for i in range(e):
        r4[i, i * r : (i + 1) * r] = 1.0

    shared = {"wlin": wlin, "ar": ar, "bf": bf}
    in_maps = []
    for c in range(n_cores):
        sh = x_flat[c * tpc : (c + 1) * tpc]
        xTc = np.ascontiguousarray(sh.T).astype(NP_MMD)  # [din, tpc]
        toks = c * tpc + np.arange(tpc)
        m = ((toks % S) < SPLIT).astype(f32)  # image-token mask
        mask_pc = np.ascontiguousarray(m.reshape(nt, 128).T)  # [128, nt]
        bb = (
            m[:, None] * np.asarray(b_router_img, f32)[None, :]
            + (1.0 - m[:, None]) * np.asarray(b_router_text, f32)[None, :]
        )  # [tpc, e]
        bbl_pc = np.ascontiguousarray(
            bb.reshape(nt, 128, e).transpose(1, 0, 2)
        ).reshape(128, nt * e)
        cfc = np.zeros((128, _CW), f32)
        cfc[:, _CO_ID : _CO_ID + 128] = np.eye(128, dtype=f32)
        cfc[:, _CO_MASK : _CO_MASK + nt] = mask_pc
        cfc[:, _CO_BBL : _CO_BBL + nt * e] = bbl_pc
        cfc[0:e, _CO_R4 : _CO_R4 + er] = r4
        in_maps.append({"xT": xTc, "cf": cfc, **shared})
    return in_maps


_prog_cache = {}


def _get_program():
    key = (D_IN, D_OUT, TPC)
    if key not in _prog_cache:
        _prog_cache[key] = build_program(D_IN, D_OUT, TPC)
    return _prog_cache[key]


def kernel(
    x, w_base, b_base, w_router_img, b_router_img, w_router_text,
    b_router_text, lora_A, lora_B,
):
    x = np.asarray(x, dtype=np.float32)
    x_flat = np.ascontiguousarray(x.reshape(TOK, D_IN))
    in_maps = pack_inputs(
        x_flat, np.asarray(w_base, np.float32), np.asarray(b_base, np.float32),
        np.asarray(w_router_img, np.float32), np.asarray(b_router_img, np.float32),
        np.asarray(w_router_text, np.float32), np.asarray(b_router_text, np.float32),
        np.asarray(lora_A, np.float32), np.asarray(lora_B, np.float32),
        N_CORES,
    )
    nc = _get_program()
    res = run_bass_kernel_spmd(nc, in_maps, core_ids=list(range(N_CORES)))
    out = np.empty((TOK, D_OUT), np.float32)
    for c in range(N_CORES):
        out[c * TPC : (c + 1) * TPC, :] = res.results[c]["outT"].T
    return out.reshape(B, S, D_OUT)


# revision 7
# speedup vs baseline: 9.8319x; 9.7725x over previous
"""Trainium2 Bass kernel for nn_Linear_67070209294813 (moe_routing).

Computes, for x:[B,S,Din] f32:
    base = x @ w_base.T + b_base
    gate = softmax(blend(x @ w_router_{img,text}.T + b_router), axis=E)
    h    = einsum("td,erd->ter", x, lora_A) * gate
    out  = base + einsum("ter,eor->to", h, lora_B) * SCALING

Strategy: data-parallel over the 8192 tokens across 8 NeuronCores (1024
tokens/core).  Per core one GEMM out^T[dout, tok] = sum_k wT-tile.T @
xT[k, tok] with the LoRA rank-65 (64 rank dims + 1 bias row) matmul
accumulated into the same PSUM banks, so bias add and the base+lora sum
cost nothing.  Inputs are cast to bf16 (fp32 PSUM accumulation) which
halves HBM traffic at identical PE throughput; the base weight is
repacked on the host into one contiguous [128, din] block per output
tile so each weight load is a single large DMA.  The router/LoRA-A
matmuls are interleaved into the first base k-loop so the PE paces with
the x-load DMAs; softmax runs in token-partition layout via two tiny PE
transposes; the gate is expanded over the 16 ranks of each expert with
a 0/1 replication matmul.

All per-core operands are packed into ONE flat bf16 DRAM blob (and one
f32 output): the host-side execution path has a fixed per-argument
dispatch cost (~1ms/arg/call) that dwarfs the kernel, so argument count
is minimized.
"""

import sys

sys.path.insert(0, "/opt/trn_rl_repo")

import numpy as np

import concourse.bass as bass  # noqa: F401  (bass must import before tile)
import concourse.mybir as mybir
import concourse.tile as tile
from concourse import bacc
from concourse.bass_utils import run_bass_kernel_spmd

B, S, D_IN, D_OUT = 4, 2048, 4096, 4096
R, E, SPLIT = 16, 4, 32
SCALING = 32.0 / 16.0
N_CORES = 8
TOK = B * S
TPC = TOK // N_CORES  # tokens per core
ER = E * R  # 64 rank dims across experts

F32 = mybir.dt.float32
MMD = mybir.dt.bfloat16
NP_MMD = mybir.dt.np(MMD)
AF = mybir.ActivationFunctionType

_NK, _NM, _NT = D_IN // 128, D_OUT // 128, TPC // 128
_WR = 2 * E  # router logit columns (img then text)
_HC = ER + _WR  # 72: lora-A ranks + both routers

# merged const-block column offsets (bf16, exact 0/1 except tiny biases):
# ident | mask | bbl | r4
_CO_ID, _CO_MASK, _CO_BBL, _CO_R4 = 0, 128, 128 + _NT, 128 + _NT + _NT * E
_CW = _CO_R4 + ER

# flat-blob element offsets (bf16)
_O_XT = 0
_O_WLIN = _O_XT + D_IN * TPC
_O_AR = _O_WLIN + D_OUT * D_IN
_O_BF = _O_AR + D_IN * _HC
_O_CF = _O_BF + (ER + 1) * D_OUT
_BLOB = _O_CF + 128 * _CW


def build_program(din, dout, tpc):
    """Emit + compile the per-core Tile program. Returns the Bacc object."""
    nk = din // 128  # k tiles (contraction)
    nm = dout // 128  # output-row tiles
    nt = tpc // 128  # token chunks (for the tiny softmax transposes)
    # moving-dim slices of the token axis (one PSUM bank = 512 fp32)
    n_sl = [(i, min(512, tpc - i)) for i in range(0, tpc, 512)]
    wr = _WR
    hcols = _HC
    xg = 4  # k-tiles per x-load DMA

    nc = bacc.Bacc(
        "TRN2", target_bir_lowering=False, debug=False, enable_partition_id=False
    )

    blob = nc.dram_tensor("blob", [_BLOB], MMD, kind="ExternalInput").ap()
    outT = nc.dram_tensor("outT", [dout, tpc], F32, kind="ExternalOutput").ap()

    def wlin_m(m):
        o = _O_WLIN + m * 128 * nk * 128
        return blob[o : o + 128 * nk * 128].rearrange("(p c) -> p c", p=128)

    with tile.TileContext(nc) as tc:
        with (
            tc.tile_pool(name="big", bufs=1) as big,
            tc.tile_pool(name="const", bufs=1) as const,
            tc.tile_pool(name="wp", bufs=2) as wp,
            tc.tile_pool(name="outp", bufs=2) as outp,
            tc.tile_pool(name="small", bufs=1) as small,
            tc.tile_pool(name="ps_main", bufs=2, space="PSUM") as ps_main,
            tc.tile_pool(name="ps_h", bufs=1, space="PSUM") as ps_h,
            tc.tile_pool(name="ps_t", bufs=2, space="PSUM") as ps_t,
        ):
            # ---- constants + x load -------------------------------------
            # first x chunk and the m=0 weight block lead the DMA queue so
            # the PE can start as soon as they land.
            xt = big.tile([128, nk * tpc], MMD)
            wt0 = wp.tile([128, nk * 128], MMD, tag="w")
            for g in range(nk // xg):
                o = _O_XT + g * xg * 128 * tpc
                nc.sync.dma_start(
                    xt[:, g * xg * tpc : (g + 1) * xg * tpc].rearrange(
                        "p (g t) -> p g t", t=tpc
                    ),
                    blob[o : o + xg * 128 * tpc].rearrange(
                        "(g p t) -> p g t", p=128, t=tpc
                    ),
                )
                if g == 0:
                    nc.sync.dma_start(wt0[:], wlin_m(0))
            ar_sb = const.tile([128, nk * hcols], MMD)
            nc.sync.dma_start(
                ar_sb[:].rearrange("p (k j) -> p k j", j=hcols),
                blob[_O_AR : _O_AR + din * hcols].rearrange(
                    "(k p j) -> p k j", p=128, j=hcols
                ),
            )
            bf_sb = const.tile([ER + 1, dout], MMD)
            nc.sync.dma_start(
                bf_sb[:],
                blob[_O_BF : _O_BF + (ER + 1) * dout].rearrange(
                    "(r c) -> r c", c=dout
                ),
            )
            cf_sb = const.tile([128, _CW], MMD)
            nc.sync.dma_start(
                cf_sb[:],
                blob[_O_CF : _O_CF + 128 * _CW].rearrange("(p c) -> p c", c=_CW),
            )
            # f32 casts of the tiny consts (ident/mask/r4 are exact 0/1)
            id_sb = const.tile([128, 128], F32)
            nc.vector.tensor_copy(id_sb[:], cf_sb[:, _CO_ID : _CO_ID + 128])
            mask_sb = const.tile([128, nt], F32)
            nc.vector.tensor_copy(mask_sb[:], cf_sb[:, _CO_MASK : _CO_MASK + nt])
            bbl_sb = const.tile([128, nt * E], F32)
            nc.vector.tensor_copy(bbl_sb[:], cf_sb[:, _CO_BBL : _CO_BBL + nt * E])
            r4_sb = const.tile([E, ER], F32)
            nc.vector.tensor_copy(r4_sb[:], cf_sb[0:E, _CO_R4 : _CO_R4 + ER])

            def base_kloop(m, also_ph=None, wt=None):
                ps = ps_main.tile([128, tpc], F32, tag="ps")
                if wt is None:
                    wt = wp.tile([128, nk * 128], MMD, tag="w")
                    nc.sync.dma_start(wt[:], wlin_m(m))
                for k in range(nk):
                    for o, w_ in n_sl:
                        nc.tensor.matmul(
                            ps[:, o : o + w_],
                            wt[:, k * 128 : (k + 1) * 128],
                            xt[:, k * tpc + o : k * tpc + o + w_],
                            start=(k == 0),
                            stop=False,
                        )
                        if also_ph is not None:
                            nc.tensor.matmul(
                                also_ph[:, o : o + w_],
                                ar_sb[:, k * hcols : (k + 1) * hcols],
                                xt[:, k * tpc + o : k * tpc + o + w_],
                                start=(k == 0),
                                stop=(k == nk - 1),
                            )
                return ps

            def lora_tail(m, ps, hw):
                for o, w_ in n_sl:
                    nc.tensor.matmul(
                        ps[:, o : o + w_],
                        bf_sb[:, m * 128 : (m + 1) * 128],
                        hw[:, o : o + w_],
                        start=False,
                        stop=True,
                    )
                ot = outp.tile([128, tpc], F32, tag="o")
                nc.scalar.copy(ot[:], ps[:])
                nc.sync.dma_start(outT[m * 128 : (m + 1) * 128, :], ot[:])

            # m=0 base matmuls with the router/lora-A matmuls interleaved so
            # the PE paces with the x-load DMAs; its lora tail is emitted
            # after gating.
            ph = ps_h.tile([hcols, tpc], F32, tag="h")
            ps0 = base_kloop(0, also_ph=ph, wt=wt0)

            hT = small.tile([hcols, tpc], F32)
            nc.vector.tensor_copy(hT[:], ph[:])
            lgT = small.tile([wr, tpc], F32)
            # partition-moving copy (rows ER..ER+wr -> 0..wr) must be a DMA
            nc.sync.dma_start(lgT[:], hT[ER : ER + wr, :])

            # ---- gating: softmax over E in token-partition layout -------
            lg = small.tile([128, nt * wr], F32)
            for t in range(nt):
                pt = ps_t.tile([128, wr], F32, tag="tp")
                nc.tensor.transpose(
                    pt[:], lgT[:, t * 128 : (t + 1) * 128], id_sb[0:wr, 0:wr]
                )
                nc.vector.tensor_copy(lg[:, t * wr : (t + 1) * wr], pt[:])
            lg3 = lg[:].rearrange("p (t j) -> p t j", j=wr)
            l_img, l_text = lg3[:, :, 0:E], lg3[:, :, E : 2 * E]
            g = small.tile([128, nt * E], F32)
            g3 = g[:].rearrange("p (t e) -> p t e", e=E)
            mb = mask_sb[:, :, None].broadcast_to([128, nt, E])
            nc.vector.tensor_sub(g3, l_img, l_text)
            nc.vector.tensor_mul(g3, g3, mb)
            nc.vector.tensor_add(g3, g3, l_text)
            nc.vector.tensor_add(g[:], g[:], bbl_sb[:])
            nc.scalar.activation(g[:], g[:], AF.Exp)
            zt = small.tile([128, nt], F32)
            nc.vector.reduce_sum(zt[:], g3, axis=mybir.AxisListType.X)
            nc.vector.reciprocal(zt[:], zt[:])
            nc.vector.tensor_mul(g3, g3, zt[:, :, None].broadcast_to([128, nt, E]))
            # gate back to [E, tok] layout, then expand across the 16 ranks
            g4 = small.tile([E, tpc], F32)
            for t in range(nt):
                pt = ps_t.tile([E, 128], F32, tag="tp")
                nc.tensor.transpose(pt[:], g[:, t * E : (t + 1) * E], id_sb[:, :])
                nc.vector.tensor_copy(g4[:, t * 128 : (t + 1) * 128], pt[:])
            pgr = ps_h.tile([ER, tpc], F32, tag="h")
            for o, w_ in n_sl:
                nc.tensor.matmul(
                    pgr[:, o : o + w_], r4_sb[:], g4[:, o : o + w_],
                    start=True, stop=True,
                )
            hw = small.tile([ER + 1, tpc], MMD)
            nc.vector.memset(hw[ER : ER + 1, :], 1.0)
            nc.vector.tensor_mul(hw[0:ER, :], hT[0:ER, :], pgr[:])

            # ---- main GEMM over output-row tiles ------------------------
            lora_tail(0, ps0, hw)
            for m in range(1, nm):
                ps = base_kloop(m)
                lora_tail(m, ps, hw)

    nc.compile()
    return nc


def pack_inputs(
    x_flat, w_base, b_base, w_router_img, b_router_img, w_router_text,
    b_router_text, lora_A, lora_B, n_cores,
):
    """Host-side marshalling into the per-core flat DRAM blob."""
    tok, din = x_flat.shape
    dout = w_base.shape[0]
    tpc = tok // n_cores
    nk, nm, nt = din // 128, dout // 128, tpc // 128
    e, r = lora_A.shape[0], lora_A.shape[1]
    er = e * r

    f32 = np.float32
    # wlin[m, p, k*128+c] = w_base[m*128+c, k*128+p]
    wlin = np.ascontiguousarray(
        np.asarray(w_base, f32).reshape(nm, 128, nk, 128).transpose(0, 3, 2, 1)
    ).astype(NP_MMD).reshape(-1)
    ar = np.concatenate(
        [lora_A.reshape(er, din).T, w_router_img.T, w_router_text.T], axis=1
    ).astype(NP_MMD).reshape(-1)  # [din, er + 2e]
    bfm = (lora_B.transpose(0, 2, 1).reshape(er, dout) * SCALING).astype(f32)
    bf = np.concatenate(
        [bfm, np.asarray(b_base, f32).reshape(1, dout)], axis=0
    ).astype(NP_MMD).reshape(-1)

    r4 = np.zeros((e, er), f32)
    for i in range(e):
        r4[i, i * r : (i + 1) * r] = 1.0

    in_maps = []
    for c in range(n_cores):
        sh = x_flat[c * tpc : (c + 1) * tpc]
        xTc = np.ascontiguousarray(sh.T).astype(NP_MMD)  # [din, tpc]
        toks = c * tpc + np.arange(tpc)
        m = ((toks % S) < SPLIT).astype(f32)  # image-token mask
        mask_pc = np.ascontiguousarray(m.reshape(nt, 128).T)  # [128, nt]
        bb = (
            m[:, None] * np.asarray(b_router_img, f32)[None, :]
            + (1.0 - m[:, None]) * np.asarray(b_router_text, f32)[None, :]
        )  # [tpc, e]
        bbl_pc = np.ascontiguousarray(
            bb.reshape(nt, 128, e).transpose(1, 0, 2)
        ).reshape(128, nt * e)
        cfc = np.zeros((128, _CW), f32)
        cfc[:, _CO_ID : _CO_ID + 128] = np.eye(128, dtype=f32)
        cfc[:, _CO_MASK : _CO_MASK + nt] = mask_pc
        cfc[:, _CO_BBL : _CO_BBL + nt * e] = bbl_pc
        cfc[0:e, _CO_R4 : _CO_R4 + er] = r4

        blob = np.empty(_BLOB, NP_MMD)
        blob[_O_XT : _O_XT + din * tpc] = xTc.reshape(-1)
        blob[_O_WLIN : _O_WLIN + dout * din] = wlin
        blob[_O_AR : _O_AR + din * _HC] = ar
        blob[_O_BF : _O_BF + (er + 1) * dout] = bf
        blob[_O_CF : _O_CF + 128 * _CW] = cfc.astype(NP_MMD).reshape(-1)
        in_maps.append({"blob": blob})
    return in_maps


_prog_cache = {}


def _get_program():
    key = (D_IN, D_OUT, TPC)
    if key not in _prog_cache:
        _prog_cache[key] = build_program(D_IN, D_OUT, TPC)
    return _prog_cache[key]


def kernel(
    x, w_base, b_base, w_router_img, b_router_img, w_router_text,
    b_router_text, lora_A, lora_B,
):
    x = np.asarray(x, dtype=np.float32)
    x_flat = np.ascontiguousarray(x.reshape(TOK, D_IN))
    in_maps = pack_inputs(
        x_flat, np.asarray(w_base, np.float32), np.asarray(b_base, np.float32),
        np.asarray(w_router_img, np.float32), np.asarray(b_router_img, np.float32),
        np.asarray(w_router_text, np.float32), np.asarray(b_router_text, np.float32),
        np.asarray(lora_A, np.float32), np.asarray(lora_B, np.float32),
        N_CORES,
    )
    nc = _get_program()
    res = run_bass_kernel_spmd(nc, in_maps, core_ids=list(range(N_CORES)))
    out = np.empty((TOK, D_OUT), np.float32)
    for c in range(N_CORES):
        out[c * TPC : (c + 1) * TPC, :] = res.results[c]["outT"].T
    return out.reshape(B, S, D_OUT)


# revision 9
# speedup vs baseline: 9.8956x; 1.0065x over previous
"""Trainium2 Bass kernel for nn_Linear_67070209294813 (moe_routing).

Computes, for x:[B,S,Din] f32:
    base = x @ w_base.T + b_base
    gate = softmax(blend(x @ w_router_{img,text}.T + b_router), axis=E)
    h    = einsum("td,erd->ter", x, lora_A) * gate
    out  = base + einsum("ter,eor->to", h, lora_B) * SCALING

Strategy: data-parallel over the 8192 tokens across 8 NeuronCores (1024
tokens/core).  Per core one GEMM out^T[dout, tok] = sum_k wT-tile.T @
xT[k, tok] with the LoRA rank-65 (64 rank dims + 1 bias row) matmul
accumulated into the same PSUM banks, so bias add and the base+lora sum
cost nothing.  Inputs are cast to bf16 (fp32 PSUM accumulation) which
halves HBM traffic at identical PE throughput; the base weight is
repacked on the host into one contiguous [128, din] block per output
tile so each weight load is a single large DMA.  The router/LoRA-A
matmuls are interleaved into the first base k-loop so the PE paces with
the x-load DMAs; softmax runs in token-partition layout via two tiny PE
transposes; the gate is expanded over the 16 ranks of each expert with
a 0/1 replication matmul.

All per-core operands are packed into ONE flat bf16 DRAM blob (and one
f32 output): the host-side execution path has a fixed per-argument
dispatch cost (~1ms/arg/call) that dwarfs the kernel, so argument count
is minimized.
"""

import sys

sys.path.insert(0, "/opt/trn_rl_repo")

import numpy as np

import concourse.bass as bass  # noqa: F401  (bass must import before tile)
import concourse.mybir as mybir
import concourse.tile as tile
from concourse import bacc
from concourse.bass_utils import run_bass_kernel_spmd

B, S, D_IN, D_OUT = 4, 2048, 4096, 4096
R, E, SPLIT = 16, 4, 32
SCALING = 32.0 / 16.0
N_CORES = 8
TOK = B * S
TPC = TOK // N_CORES  # tokens per core
ER = E * R  # 64 rank dims across experts

F32 = mybir.dt.float32
MMD = mybir.dt.bfloat16
NP_MMD = mybir.dt.np(MMD)
AF = mybir.ActivationFunctionType

_NK, _NM, _NT = D_IN // 128, D_OUT // 128, TPC // 128
_WR = 2 * E  # router logit columns (img then text)
_HC = ER + _WR  # 72: lora-A ranks + both routers

# merged const-block column offsets (bf16, exact 0/1 except tiny biases):
# ident | mask | bbl | r4
_CO_ID, _CO_MASK, _CO_BBL, _CO_R4 = 0, 128, 128 + _NT, 128 + _NT + _NT * E
_CW = _CO_R4 + ER

# flat-blob element offsets (bf16)
_O_XT = 0
_O_WLIN = _O_XT + D_IN * TPC
_O_AR = _O_WLIN + D_OUT * D_IN
_O_BF = _O_AR + D_IN * _HC
_O_CF = _O_BF + (ER + 1) * D_OUT
_BLOB = _O_CF + 128 * _CW


def build_program(din, dout, tpc):
    """Emit + compile the per-core Tile program. Returns the Bacc object."""
    nk = din // 128  # k tiles (contraction)
    nm = dout // 128  # output-row tiles
    nt = tpc // 128  # token chunks (for the tiny softmax transposes)
    # moving-dim slices of the token axis (one PSUM bank = 512 fp32)
    n_sl = [(i, min(512, tpc - i)) for i in range(0, tpc, 512)]
    wr = _WR
    hcols = _HC
    xg = 4  # k-tiles per x-load DMA

    nc = bacc.Bacc(
        "TRN2", target_bir_lowering=False, debug=False, enable_partition_id=False
    )

    blob = nc.dram_tensor("blob", [_BLOB], MMD, kind="ExternalInput").ap()
    outT = nc.dram_tensor("outT", [dout, tpc], F32, kind="ExternalOutput").ap()

    def wlin_m(m):
        o = _O_WLIN + m * 128 * nk * 128
        return blob[o : o + 128 * nk * 128].rearrange("(p c) -> p c", p=128)

    with tile.TileContext(nc) as tc:
        with (
            tc.tile_pool(name="big", bufs=1) as big,
            tc.tile_pool(name="const", bufs=1) as const,
            tc.tile_pool(name="wp", bufs=2) as wp,
            tc.tile_pool(name="outp", bufs=2) as outp,
            tc.tile_pool(name="small", bufs=1) as small,
            tc.tile_pool(name="ps_main", bufs=2, space="PSUM") as ps_main,
            tc.tile_pool(name="ps_h", bufs=1, space="PSUM") as ps_h,
            tc.tile_pool(name="ps_t", bufs=2, space="PSUM") as ps_t,
        ):
            # ---- constants + x load -------------------------------------
            # first x chunk and the m=0 weight block lead the DMA queue so
            # the PE can start as soon as they land.
            xt = big.tile([128, nk * tpc], MMD)
            wt0 = wp.tile([128, nk * 128], MMD, tag="w")
            for g in range(nk // xg):
                o = _O_XT + g * xg * 128 * tpc
                nc.sync.dma_start(
                    xt[:, g * xg * tpc : (g + 1) * xg * tpc].rearrange(
                        "p (g t) -> p g t", t=tpc
                    ),
                    blob[o : o + xg * 128 * tpc].rearrange(
                        "(g p t) -> p g t", p=128, t=tpc
                    ),
                )
                if g == 0:
                    nc.sync.dma_start(wt0[:], wlin_m(0))
            ar_sb = const.tile([128, nk * hcols], MMD)
            nc.sync.dma_start(
                ar_sb[:].rearrange("p (k j) -> p k j", j=hcols),
                blob[_O_AR : _O_AR + din * hcols].rearrange(
                    "(k p j) -> p k j", p=128, j=hcols
                ),
            )
            bf_sb = const.tile([ER + 1, dout], MMD)
            nc.sync.dma_start(
                bf_sb[:],
                blob[_O_BF : _O_BF + (ER + 1) * dout].rearrange(
                    "(r c) -> r c", c=dout
                ),
            )
            cf_sb = const.tile([128, _CW], MMD)
            nc.sync.dma_start(
                cf_sb[:],
                blob[_O_CF : _O_CF + 128 * _CW].rearrange("(p c) -> p c", c=_CW),
            )
            # f32 casts of the tiny consts (ident/mask/r4 are exact 0/1)
            id_sb = const.tile([128, 128], F32)
            nc.vector.tensor_copy(id_sb[:], cf_sb[:, _CO_ID : _CO_ID + 128])
            mask_sb = const.tile([128, nt], F32)
            nc.vector.tensor_copy(mask_sb[:], cf_sb[:, _CO_MASK : _CO_MASK + nt])
            bbl_sb = const.tile([128, nt * E], F32)
            nc.vector.tensor_copy(bbl_sb[:], cf_sb[:, _CO_BBL : _CO_BBL + nt * E])
            r4_sb = const.tile([E, ER], F32)
            nc.vector.tensor_copy(r4_sb[:], cf_sb[0:E, _CO_R4 : _CO_R4 + ER])

            def base_kloop(m, also_ph=None, wt=None):
                ps = ps_main.tile([128, tpc], F32, tag="ps")
                if wt is None:
                    wt = wp.tile([128, nk * 128], MMD, tag="w")
                    nc.sync.dma_start(wt[:], wlin_m(m))
                for k in range(nk):
                    for o, w_ in n_sl:
                        nc.tensor.matmul(
                            ps[:, o : o + w_],
                            wt[:, k * 128 : (k + 1) * 128],
                            xt[:, k * tpc + o : k * tpc + o + w_],
                            start=(k == 0),
                            stop=False,
                        )
                        if also_ph is not None:
                            nc.tensor.matmul(
                                also_ph[:, o : o + w_],
                                ar_sb[:, k * hcols : (k + 1) * hcols],
                                xt[:, k * tpc + o : k * tpc + o + w_],
                                start=(k == 0),
                                stop=(k == nk - 1),
                            )
                return ps

            def lora_tail(m, ps, hw):
                for o, w_ in n_sl:
                    nc.tensor.matmul(
                        ps[:, o : o + w_],
                        bf_sb[:, m * 128 : (m + 1) * 128],
                        hw[:, o : o + w_],
                        start=False,
                        stop=True,
                    )
                ot = outp.tile([128, tpc], F32, tag="o")
                nc.scalar.copy(ot[:], ps[:])
                nc.sync.dma_start(outT[m * 128 : (m + 1) * 128, :], ot[:])

            # m=0 base matmuls with the router/lora-A matmuls interleaved so
            # the PE paces with the x-load DMAs; its lora tail is emitted
            # after gating.
            ph = ps_h.tile([hcols, tpc], F32, tag="h")
            ps0 = base_kloop(0, also_ph=ph, wt=wt0)

            hT = small.tile([hcols, tpc], F32)
            nc.vector.tensor_copy(hT[:], ph[:])
            lgT = small.tile([wr, tpc], F32)
            # partition-moving copy (rows ER..ER+wr -> 0..wr) must be a DMA
            nc.sync.dma_start(lgT[:], hT[ER : ER + wr, :])

            # ---- gating: softmax over E in token-partition layout -------
            lg = small.tile([128, nt * wr], F32)
            for t in range(nt):
                pt = ps_t.tile([128, wr], F32, tag="tp")
                nc.tensor.transpose(
                    pt[:], lgT[:, t * 128 : (t + 1) * 128], id_sb[0:wr, 0:wr]
                )
                nc.vector.tensor_copy(lg[:, t * wr : (t + 1) * wr], pt[:])
            lg3 = lg[:].rearrange("p (t j) -> p t j", j=wr)
            l_img, l_text = lg3[:, :, 0:E], lg3[:, :, E : 2 * E]
            g = small.tile([128, nt * E], F32)
            g3 = g[:].rearrange("p (t e) -> p t e", e=E)
            mb = mask_sb[:, :, None].broadcast_to([128, nt, E])
            nc.vector.tensor_sub(g3, l_img, l_text)
            nc.vector.tensor_mul(g3, g3, mb)
            nc.vector.tensor_add(g3, g3, l_text)
            nc.vector.tensor_add(g[:], g[:], bbl_sb[:])
            nc.scalar.activation(g[:], g[:], AF.Exp)
            zt = small.tile([128, nt], F32)
            nc.vector.reduce_sum(zt[:], g3, axis=mybir.AxisListType.X)
            nc.vector.reciprocal(zt[:], zt[:])
            nc.vector.tensor_mul(g3, g3, zt[:, :, None].broadcast_to([128, nt, E]))
            # gate back to [E, tok] layout, then expand across the 16 ranks
            g4 = small.tile([E, tpc], F32)
            for t in range(nt):
                pt = ps_t.tile([E, 128], F32, tag="tp")
                nc.tensor.transpose(pt[:], g[:, t * E : (t + 1) * E], id_sb[:, :])
                nc.vector.tensor_copy(g4[:, t * 128 : (t + 1) * 128], pt[:])
            pgr = ps_h.tile([ER, tpc], F32, tag="h")
            for o, w_ in n_sl:
                nc.tensor.matmul(
                    pgr[:, o : o + w_], r4_sb[:], g4[:, o : o + w_],
                    start=True, stop=True,
                )
            hw = small.tile([ER + 1, tpc], MMD)
            nc.vector.memset(hw[ER : ER + 1, :], 1.0)
            nc.vector.tensor_mul(hw[0:ER, :], hT[0:ER, :], pgr[:])

            # ---- main GEMM over output-row tiles ------------------------
            lora_tail(0, ps0, hw)
            for m in range(1, nm):
                ps = base_kloop(m)
                lora_tail(m, ps, hw)

    nc.compile()
    return nc


def pack_inputs(
    x_flat, w_base, b_base, w_router_img, b_router_img, w_router_text,
    b_router_text, lora_A, lora_B, n_cores,
):
    """Host-side marshalling into the per-core flat DRAM blob."""
    tok, din = x_flat.shape
    dout = w_base.shape[0]
    tpc = tok // n_cores
    nk, nm, nt = din // 128, dout // 128, tpc // 128
    e, r = lora_A.shape[0], lora_A.shape[1]
    er = e * r

    f32 = np.float32
    # wlin[m, p, k*128+c] = w_base[m*128+c, k*128+p]
    wlin = np.ascontiguousarray(
        np.asarray(w_base, f32).reshape(nm, 128, nk, 128).transpose(0, 3, 2, 1)
    ).astype(NP_MMD).reshape(-1)
    ar = np.concatenate(
        [lora_A.reshape(er, din).T, w_router_img.T, w_router_text.T], axis=1
    ).astype(NP_MMD).reshape(-1)  # [din, er + 2e]
    bfm = (lora_B.transpose(0, 2, 1).reshape(er, dout) * SCALING).astype(f32)
    bf = np.concatenate(
        [bfm, np.asarray(b_base, f32).reshape(1, dout)], axis=0
    ).astype(NP_MMD).reshape(-1)

    r4 = np.zeros((e, er), f32)
    for i in range(e):
        r4[i, i * r : (i + 1) * r] = 1.0

    in_maps = []
    for c in range(n_cores):
        sh = x_flat[c * tpc : (c + 1) * tpc]
        xTc = np.ascontiguousarray(sh.T).astype(NP_MMD)  # [din, tpc]
        toks = c * tpc + np.arange(tpc)
        m = ((toks % S) < SPLIT).astype(f32)  # image-token mask
        mask_pc = np.ascontiguousarray(m.reshape(nt, 128).T)  # [128, nt]
        bb = (
            m[:, None] * np.asarray(b_router_img, f32)[None, :]
            + (1.0 - m[:, None]) * np.asarray(b_router_text, f32)[None, :]
        )  # [tpc, e]
        bbl_pc = np.ascontiguousarray(
            bb.reshape(nt, 128, e).transpose(1, 0, 2)
        ).reshape(128, nt * e)
        cfc = np.zeros((128, _CW), f32)
        cfc[:, _CO_ID : _CO_ID + 128] = np.eye(128, dtype=f32)
        cfc[:, _CO_MASK : _CO_MASK + nt] = mask_pc
        cfc[:, _CO_BBL : _CO_BBL + nt * e] = bbl_pc
        cfc[0:e, _CO_R4 : _CO_R4 + er] = r4

        blob = np.empty(_BLOB, NP_MMD)
        blob[_O_XT : _O_XT + din * tpc] = xTc.reshape(-1)
        blob[_O_WLIN : _O_WLIN + dout * din] = wlin
        blob[_O_AR : _O_AR + din * _HC] = ar
        blob[_O_BF : _O_BF + (er + 1) * dout] = bf
        blob[_O_CF : _O_CF + 128 * _CW] = cfc.astype(NP_MMD).reshape(-1)
        in_maps.append({"blob": blob})
    return in_maps


_prog_cache = {}


def _get_program():
    key = (D_IN, D_OUT, TPC)
    if key not in _prog_cache:
        _prog_cache[key] = build_program(D_IN, D_OUT, TPC)
    return _prog_cache[key]


def kernel(
    x, w_base, b_base, w_router_img, b_router_img, w_router_text,
    b_router_text, lora_A, lora_B,
):
    x = np.asarray(x, dtype=np.float32)
    x_flat = np.ascontiguousarray(x.reshape(TOK, D_IN))
    in_maps = pack_inputs(
        x_flat, np.asarray(w_base, np.float32), np.asarray(b_base, np.float32),
        np.asarray(w_router_img, np.float32), np.asarray(b_router_img, np.float32),
        np.asarray(w_router_text, np.float32), np.asarray(b_router_text, np.float32),
        np.asarray(lora_A, np.float32), np.asarray(lora_B, np.float32),
        N_CORES,
    )
    nc = _get_program()
    res = run_bass_kernel_spmd(nc, in_maps, core_ids=list(range(N_CORES)))
    out = np.empty((TOK, D_OUT), np.float32)
    for c in range(N_CORES):
        out[c * TPC : (c + 1) * TPC, :] = res.results[c]["outT"].T
    return out.reshape(B, S, D_OUT)
